# revision 5
# baseline (speedup 1.0000x reference)
"""Trainium2 Bass kernel v3: windowed-LSTM local attention + linear head.

vs v2:
- Gate sigmoids batched: the i/f/o pre-activations for one d-chunk go into a
  single 3-bank PSUM tile ([128, 3, 512]) so ONE ACT instruction drains all
  three (ACT ops per step: 30 -> 18). The g-gate (tanh) keeps its own bank.
- Step-0 reads the i and o rows of P with one strided 2-row ACT op.
- Windows processed in order (7, 5, 3): the long window's 5 step-gaps absorb
  the other windows' input projections as PE filler; then the 5- and
  3-windows' recurrences are interleaved so each window's step tail hides
  under the other's ACT work.
- fp8(e4m3) DoubleRow matmuls (weights x16, h x4 to dodge fp8 subnormals),
  P stored fp8 x16, injected via 4*I fp8 matmul; ACT un-scales by 1/64.
- Attention/head in [9, L] space: logits = lw@x + sum_k softmax_k (*) (lw@h_k).
"""

import math
import numpy as np
import ml_dtypes

import concourse.bacc as bacc
import concourse.bass as bass
import concourse.tile as tile
from concourse import mybir
from concourse import bass_utils
from concourse.alu_op_type import AluOpType

B, L, D = 8, 512, 768
NL = 9
WINDOWS = (3, 5, 7)
NW = len(WINDOWS)
G4 = 4 * D          # 3072
P = 128
ND = D // P         # 6 k-chunks of the contraction dim
NM = G4 // P        # 24 gate-chunks
NJ = ND // 2        # 3 DoubleRow k-pairs
N_CORES = 8

SW = 16.0           # host weight scale (fp8 subnormal avoidance)
SH = 4.0            # h state scale
SINJ = SW * SH      # PSUM scale of hidden products = 64

F32 = mybir.dt.float32
BF16 = mybir.dt.bfloat16
F8 = mybir.dt.float8e4
AF = mybir.ActivationFunctionType
DR = mybir.MatmulPerfMode.DoubleRow

NPF8 = mybir.dt.np(F8)
NPBF = mybir.dt.np(BF16)


def _emit(tc, io):
    nc = tc.nc
    from contextlib import ExitStack

    with ExitStack() as ctx:
        const = ctx.enter_context(tc.tile_pool(name="const", bufs=1))
        wpool = ctx.enter_context(tc.tile_pool(name="wpool", bufs=1))
        ppool = ctx.enter_context(tc.tile_pool(name="ppool", bufs=1))
        state = ctx.enter_context(tc.tile_pool(name="state", bufs=1))
        gpool = ctx.enter_context(tc.tile_pool(name="gates", bufs=1))
        tpool = ctx.enter_context(tc.tile_pool(name="tmp", bufs=4))
        apool = ctx.enter_context(tc.tile_pool(name="attn", bufs=1))
        psum = ctx.enter_context(tc.tile_pool(name="psum", bufs=2, space="PSUM"))

        # ---- resident constants ----
        # DMA order = critical path order: xq -> wih[first] -> proj -> step0.
        FIRST = 2                     # window order: 7, then 5 & 3 interleaved
        xq = const.tile([P, ND, L], F8, tag="xq")
        nc.sync.dma_start(xq, io["xq"].ap())
        bias_sb = const.tile([P, NW, NM], F32, tag="bias")
        nc.sync.dma_start(bias_sb, io["bias16"].ap())
        wi0 = wpool.tile([P, ND, G4], F8, tag="wi")
        for j in range(NJ):  # split so early proj matmuls start sooner
            nc.sync.dma_start(wi0[:, 2 * j:2 * j + 2, :],
                              io["wih"].ap()[FIRST, :, 2 * j:2 * j + 2, :])
        ident4 = const.tile([P, P], F8, tag="ident4")
        nc.sync.dma_start(ident4, io["ident4"].ap())
        whh = []
        for k in range(NW):
            t = wpool.tile([P, ND, G4], F8, tag=f"whh{k}")
            whh.append(t)
        nc.sync.dma_start(whh[2], io["whh"].ap()[2])
        xb = const.tile([P, ND, L], BF16, tag="xb")
        nc.sync.dma_start(xb, io["xb"].ap())
        nc.sync.dma_start(whh[1], io["whh"].ap()[1])
        nc.sync.dma_start(whh[0], io["whh"].ap()[0])
        lwt = const.tile([P, ND, NL], BF16, tag="lwt")
        nc.sync.dma_start(lwt, io["lwt"].ap())
        lb_sb = const.tile([NL, 1], F32, tag="lb")
        nc.sync.dma_start(lb_sb, io["lb"].ap())
        ones_col = const.tile([P, 1], BF16, tag="ones_col")
        nc.vector.memset(ones_col, 1.0)
        ones9 = const.tile([1, NL], BF16, tag="ones9")
        nc.vector.memset(ones9, 1.0)

        Ps = [ppool.tile([P, NM, L], F8, tag=f"P{k}", name=f"P{k}")
              for k in range(NW)]
        h_s = []
        for k in range(NW):
            t = state.tile([P, ND, L], F8, tag=f"h{k}", name=f"h{k}")
            h_s.append(t)

        inv_sqrt_d = 1.0 / math.sqrt(D)

        # ---------- emission helpers ----------
        def emit_wi_dma(k):
            wi = wpool.tile([P, ND, G4], F8, tag="wi", name="wi")
            nc.sync.dma_start(wi, io["wih"].ap()[k])
            return wi

        def emit_proj_chunk(k, wi, m):
            """P_s[k][:, m, :] = fp8( (16*Wih)@x + 16*bias )"""
            ps = psum.tile([P, L], F32, tag="g", name="ps")
            for j in range(NJ):
                nc.tensor.matmul(
                    ps,
                    lhsT=wi[:, 2 * j:2 * j + 2, m * P:(m + 1) * P],
                    rhs=xq[:, 2 * j:2 * j + 2, :],
                    start=(j == 0),
                    stop=(j == NJ - 1),
                    perf_mode=DR,
                )
            eng = (nc.vector, nc.scalar, nc.vector)[m % 3]
            if eng is nc.scalar:
                nc.scalar.activation(Ps[k][:, m, :], ps, AF.Identity,
                                     bias=bias_sb[:, k, m:m + 1])
            else:
                eng.tensor_scalar_add(Ps[k][:, m, :], ps, bias_sb[:, k, m:m + 1])

        def emit_state_init(k, w):
            hw_ = w // 2
            c = state.tile([P, ND, L], BF16, tag="c", bufs=2, name="c")
            nc.vector.memset(c[:, :, 0:hw_], 0.0)
            nc.vector.memset(h_s[k][:, :, 0:hw_], 0.0)
            return c

        def emit_cell(k, c, dc, s, e, n, ia, ga, oa, first):
            """c/h update for one d-chunk (gate APs already sliced)."""
            if first:
                nc.vector.tensor_mul(c[:, dc, s:e], ia, ga)
            else:
                ta = tpool.tile([P, L], BF16, tag="t", name="ta")
                nc.vector.tensor_mul(ta[:, :n], ia, ga)
                tb = tpool.tile([P, L], BF16, tag="t", name="tb")
                nc.vector.tensor_mul(tb[:, :n], oa_f[0], c[:, dc, s:e])
                nc.vector.tensor_add(c[:, dc, s:e], ta[:, :n], tb[:, :n])
            tch = tpool.tile([P, L], BF16, tag="tch", name="tch")
            nc.scalar.activation(tch[:, :n], c[:, dc, s:e], AF.Tanh)
            # scalar_tensor_tensor is DVE-only (Pool/GPSIMD lacks the opcode)
            nc.vector.scalar_tensor_tensor(
                h_s[k][:, dc, s:e], tch[:, :n], SH, oa,
                op0=AluOpType.mult, op1=AluOpType.mult,
            )

        oa_f = [None]  # forget-gate AP holder for emit_cell

        def emit_step0(k, w, c):
            hw_ = w // 2
            s, e = hw_, L           # off = -hw: cols [hw, L)
            n = e - s
            off = -hw_
            ifo = gpool.tile([P, 3, ND, L], BF16, tag="ifo", name="ifo")
            ga = gpool.tile([P, ND, L], BF16, tag="ga", name="ga")
            for dc in range(ND):
                # one strided 2-row op: sigmoid of the i row (dc) and o row
                # (18+dc) of P -> ifo subtiles 0 and 2
                nc.scalar.activation(
                    ifo[:, 0:3:2, dc, s:e],
                    Ps[k][:, 0 + dc:19 + dc:18, s + off:e + off],
                    AF.Sigmoid, scale=1.0 / SW)
                nc.scalar.activation(
                    ga[:, dc, s:e], Ps[k][:, 12 + dc, s + off:e + off],
                    AF.Tanh, scale=1.0 / SW)
                if dc >= 1:
                    p = dc - 1
                    emit_cell(k, c, p, s, e, n, ifo[:, 0, p, s:e],
                              ga[:, p, s:e], ifo[:, 2, p, s:e], first=True)
            p = ND - 1
            emit_cell(k, c, p, s, e, n, ifo[:, 0, p, s:e],
                      ga[:, p, s:e], ifo[:, 2, p, s:e], first=True)

        def emit_step(k, w, t, c):
            hw_ = w // 2
            off = t - hw_
            s = max(0, -off)
            e = min(L, L - off)
            n = e - s

            def fill(ps_slice, m):
                nc.tensor.matmul(
                    ps_slice,
                    lhsT=ident4[:],
                    rhs=Ps[k][:, m, s + off:e + off],
                    start=True, stop=False,
                )
                for j in range(NJ):
                    nc.tensor.matmul(
                        ps_slice,
                        lhsT=whh[k][:, 2 * j:2 * j + 2, m * P:(m + 1) * P],
                        rhs=h_s[k][:, 2 * j:2 * j + 2, s:e],
                        start=False, stop=(j == NJ - 1),
                        perf_mode=DR,
                    )

            trips, gps = {}, {}
            for dc in range(ND):
                trip = psum.tile([P, 3, L], F32, tag="g3", name="trip")
                for gi, base in enumerate((0, 6, 18)):   # i, f, o sigmoids
                    fill(trip[:, gi, s:e], base + dc)
                gp = psum.tile([P, L], F32, tag="g", name="gp")
                fill(gp[:, s:e], 12 + dc)
                trips[dc], gps[dc] = trip, gp
            ifo = gpool.tile([P, 3, ND, L], BF16, tag="ifo", name="ifo")
            ga = gpool.tile([P, ND, L], BF16, tag="ga", name="ga")
            inv = 1.0 / SINJ

            def cell(p):
                oa_f[0] = ifo[:, 1, p, s:e]      # forget gate
                emit_cell(k, c, p, s, e, n, ifo[:, 0, p, s:e],
                          ga[:, p, s:e], ifo[:, 2, p, s:e], first=False)

            for dc in range(ND):
                nc.scalar.activation(ifo[:, :, dc, s:e], trips[dc][:, :, s:e],
                                     AF.Sigmoid, scale=inv)
                nc.scalar.activation(ga[:, dc, s:e], gps[dc][:, s:e],
                                     AF.Tanh, scale=inv)
                if dc >= 1:
                    cell(dc - 1)
            cell(ND - 1)

        e_sb = {}
        y_sb = {}

        def emit_tail(k):
            """attention dot + head projection of this window's locals."""
            a_ps = psum.tile([1, L], F32, tag="g", name="a_ps")
            y_ps = psum.tile([NL, L], F32, tag="g", name="y_ps")
            for dc in range(ND):
                hb = tpool.tile([P, L], BF16, tag="hb", bufs=4, name="hb")
                nc.gpsimd.tensor_scalar_mul(hb, h_s[k][:, dc, :], 1.0 / SH)
                td = tpool.tile([P, L], BF16, tag="td", bufs=4, name="td")
                nc.vector.tensor_mul(td, xb[:, dc, :], hb[:])
                nc.tensor.matmul(a_ps, lhsT=ones_col[:], rhs=td[:],
                                 start=(dc == 0), stop=(dc == ND - 1))
                nc.tensor.matmul(y_ps, lhsT=lwt[:, dc, :], rhs=hb[:],
                                 start=(dc == 0), stop=(dc == ND - 1))
            ek = apool.tile([1, L], F32, tag=f"e{k}", name=f"e{k}")
            nc.scalar.activation(ek, a_ps, AF.Exp, scale=inv_sqrt_d)
            e_sb[k] = ek
            yk = apool.tile([NL, L], F32, tag=f"y{k}", name=f"y{k}")
            nc.vector.tensor_copy(yk, y_ps)
            y_sb[k] = yk

        pre = {}

        def emit_s1():
            s1 = apool.tile([1, L], F32, tag="sm", bufs=3, name="s1")
            nc.vector.tensor_add(s1, e_sb[2][:], e_sb[0][:])
            pre["s1"] = s1

        def emit_hp():
            hp = psum.tile([NL, L], F32, tag="g", name="hp")
            for dc in range(ND):
                nc.tensor.matmul(hp, lhsT=lwt[:, dc, :], rhs=xb[:, dc, :],
                                 start=(dc == 0), stop=(dc == ND - 1))
            # drain to SBUF at once: holding the PSUM ring slot until the
            # final add would deadlock against the later tail/wb allocations
            hp_sb = apool.tile([NL, L], F32, tag="hp_sb", name="hp_sb")
            nc.vector.tensor_copy(hp_sb, hp)
            pre["hp"] = hp_sb

        def emit_final():
            s2 = apool.tile([1, L], F32, tag="sm", bufs=3, name="s2")
            nc.vector.tensor_add(s2, pre["s1"][:], e_sb[1][:])
            r = apool.tile([1, L], F32, tag="sm", bufs=3, name="r")
            nc.vector.reciprocal(r, s2[:])
            wbs = []
            for k in range(NW):
                wn = apool.tile([1, L], BF16, tag="wn", bufs=3, name="wn")
                nc.vector.tensor_mul(wn, e_sb[k][:], r[:])
                wb = psum.tile([NL, L], F32, tag="g3", name="wb")
                nc.tensor.matmul(wb, lhsT=ones9[:], rhs=wn[:],
                                 start=True, stop=True)
                wbs.append(wb)
            m0 = apool.tile([NL, L], F32, tag="fin", bufs=3, name="m0")
            nc.vector.tensor_mul(m0, wbs[0][:], y_sb[0][:])
            m1 = apool.tile([NL, L], F32, tag="fin", bufs=3, name="m1")
            nc.vector.tensor_mul(m1, wbs[1][:], y_sb[1][:])
            a01 = apool.tile([NL, L], F32, tag="fin", bufs=3, name="a01")
            nc.vector.tensor_add(a01, m0[:], m1[:])
            m2 = apool.tile([NL, L], F32, tag="fin", bufs=3, name="m2")
            nc.vector.tensor_mul(m2, wbs[2][:], y_sb[2][:])
            a012 = apool.tile([NL, L], F32, tag="fin", bufs=3, name="a012")
            nc.vector.tensor_add(a012, a01[:], m2[:])
            af = apool.tile([NL, L], F32, tag="fin", bufs=3, name="af")
            nc.vector.tensor_add(af, a012[:], pre["hp"][:])
            logits = apool.tile([NL, L], F32, tag="fin", bufs=3, name="logits")
            nc.scalar.activation(logits, af[:], AF.Identity, bias=lb_sb[:, 0:1])
            nc.sync.dma_start(io["out"].ap(), logits[:])

        # ---------- schedule ----------
        fillers = []

        def drain_fillers(nmax):
            nonlocal fillers
            take, fillers = fillers[:nmax], fillers[nmax:]
            for f in take:
                f()

        # phase A: window 2 (w=7); its 5 step gaps absorb the other windows'
        # projections
        for m in range(NM):
            emit_proj_chunk(2, wi0, m)
        wi1 = emit_wi_dma(1)
        fillers += [(lambda mm=m: emit_proj_chunk(1, wi1, mm))
                    for m in range(NM)]
        c2 = emit_state_init(2, 7)
        emit_step0(2, 7, c2)
        queued0 = False
        for t in range(1, 7):
            if t >= 2:
                drain_fillers(8)
            if t == 2 and not queued0:
                # queue window-0 projection once wi ring frees up
                wi00 = emit_wi_dma(0)
                fillers += [(lambda mm=m: emit_proj_chunk(0, wi00, mm))
                            for m in range(NM)]
                queued0 = True
            emit_step(2, 7, t, c2)
        emit_tail(2)

        # phase B: windows 1 (w=5) and 0 (w=3) interleaved
        c1 = emit_state_init(1, 5)
        emit_step0(1, 5, c1)
        drain_fillers(len(fillers))     # finish proj(0)
        c0 = emit_state_init(0, 3)
        emit_step0(0, 3, c0)
        emit_step(1, 5, 1, c1)
        emit_step(0, 3, 1, c0)
        emit_step(1, 5, 2, c1)
        emit_step(1, 5, 3, c1)
        emit_step(0, 3, 2, c0)
        emit_tail(0)
        emit_step(1, 5, 4, c1)
        emit_hp()
        emit_tail(1)
        emit_s1()
        emit_final()


_NC_CACHE = {}


def _build_nc():
    nc = bacc.Bacc("TRN2", target_bir_lowering=False, debug=False)
    io = {
        "xq": nc.dram_tensor("xq", [P, ND, L], F8, kind="ExternalInput"),
        "xb": nc.dram_tensor("xb", [P, ND, L], BF16, kind="ExternalInput"),
        "wih": nc.dram_tensor("wih", [NW, P, ND, G4], F8, kind="ExternalInput"),
        "whh": nc.dram_tensor("whh", [NW, P, ND, G4], F8, kind="ExternalInput"),
        "bias16": nc.dram_tensor("bias16", [P, NW, NM], F32, kind="ExternalInput"),
        "lwt": nc.dram_tensor("lwt", [P, ND, NL], BF16, kind="ExternalInput"),
        "lb": nc.dram_tensor("lb", [NL, 1], F32, kind="ExternalInput"),
        "ident4": nc.dram_tensor("ident4", [P, P], F8, kind="ExternalInput"),
        "out": nc.dram_tensor("out", [NL, L], F32, kind="ExternalOutput"),
    }
    with tile.TileContext(nc) as tc:
        _emit(tc, io)
    nc.compile()
    return nc


def _get_nc():
    if "nc" not in _NC_CACHE:
        _NC_CACHE["nc"] = _build_nc()
    return _NC_CACHE["nc"]


def _in_maps(sequence_output, W_ih, W_hh, b_ih, b_hh, lin_w, lin_b):
    x = np.asarray(sequence_output, np.float32)

    def kblocked(a):   # [C, F] -> [128, C//128, F]
        C, F = a.shape
        return np.ascontiguousarray(
            a.reshape(C // P, P, F).transpose(1, 0, 2))

    wih_l = np.stack([
        kblocked(np.asarray(W_ih[k], np.float32).T * SW) for k in range(NW)
    ]).astype(NPF8)
    whh_l = np.stack([
        kblocked(np.asarray(W_hh[k], np.float32).T * SW) for k in range(NW)
    ]).astype(NPF8)
    biasc = (np.asarray(b_ih, np.float32) + np.asarray(b_hh, np.float32)) * SW
    bias16 = np.ascontiguousarray(
        biasc.reshape(NW, NM, P).transpose(2, 0, 1))     # [128, 3, 24]
    lwt = kblocked(np.asarray(lin_w, np.float32).T).astype(NPBF)  # [128, 6, 9]
    lb = np.asarray(lin_b, np.float32).reshape(NL, 1)
    ident4 = (SH * np.eye(P, dtype=np.float32)).astype(NPF8)

    maps = []
    for b in range(B):
        xT = np.ascontiguousarray(x[b].T)                # [768, 512]
        xkb = kblocked(xT)                               # [128, 6, 512]
        maps.append({
            "xq": xkb.astype(NPF8),
            "xb": xkb.astype(NPBF),
            "wih": wih_l,
            "whh": whh_l,
            "bias16": bias16,
            "lwt": lwt,
            "lb": lb,
            "ident4": ident4,
        })
    return maps


def kernel(sequence_output, W_ih, W_hh, b_ih, b_hh, lin_w, lin_b):
    nc = _get_nc()
    maps = _in_maps(sequence_output, W_ih, W_hh, b_ih, b_hh, lin_w, lin_b)
    res = bass_utils.run_bass_kernel_spmd(nc, maps, core_ids=list(range(N_CORES)))
    return np.stack(
        [np.ascontiguousarray(res.results[b]["out"].T) for b in range(B)], axis=0
    )


def run_traced(inputs, **kw):
    nc = _get_nc()
    maps = _in_maps(**inputs)
    return bass_utils.run_bass_kernel_spmd(
        nc, maps, core_ids=list(range(N_CORES)), trace=True, **kw
    )
